# revision 4
# baseline (speedup 1.0000x reference)
"""Trainium2 Bass kernel for nn_CustomActivation:

    out[b, d] = sum_k alpha[k, d % 64] * relu(x[b, d] + gamma[k, d % 64])

x: [8192, 4096] f32, alpha/gamma: [3, 64] f32.

Strategy
--------
Shard x along the FEATURE axis (columns) across 8 cores, 512 columns each,
and hand every core a TRANSPOSED, contiguous shard xT [512, 8192] (host-side
numpy transpose).  On-chip layout is then [partition = d, free = b]:

  * the per-column params alpha/gamma become per-PARTITION scalars
    ([128, 1] APs), so the whole op maps onto cheap per-partition hardware:
      - ScalarE activation:  t_k = Relu(x + g_k)        (bias = per-partition AP)
      - VectorE tensor_scalar: acc = t_0 * a_0          (fp32 2x mode)
      - VectorE scalar_tensor_tensor: acc = (t_k * a_k) + acc   (fused)
  * every DMA is fully contiguous (32KB per partition-row), full HBM BW.

Since the d-range of every 128-partition block is a multiple of 64, the
[128] param vectors (= param[k, p % 64]) are identical for all blocks/cores.
"""

import numpy as np

import concourse.bacc as bacc
import concourse.mybir as mybir
from concourse.tile import TileContext

N_CORES = 8
B, D, L = 8192, 4096, 64
DS = D // N_CORES  # 512 feature columns per core
P = 128  # SBUF partitions


def build_program(ds: int = DS, b: int = B, f_tile: int = 2048):
    """Build the SPMD Bass program one core runs on its [ds, b] shard."""
    nc = bacc.Bacc("TRN2", target_bir_lowering=False, debug=False)

    xT = nc.dram_tensor("xT", [ds, b], mybir.dt.float32, kind="ExternalInput").ap()
    av = nc.dram_tensor("av", [P, 3], mybir.dt.float32, kind="ExternalInput").ap()
    gv = nc.dram_tensor("gv", [P, 3], mybir.dt.float32, kind="ExternalInput").ap()
    oT = nc.dram_tensor("oT", [ds, b], mybir.dt.float32, kind="ExternalOutput").ap()

    n_blk = ds // P
    n_f = b // f_tile

    with TileContext(nc) as tc:
        with (
            tc.tile_pool(name="params", bufs=1) as ppool,
            tc.tile_pool(name="xin", bufs=3) as xpool,
            tc.tile_pool(name="t1", bufs=2) as t1pool,
            tc.tile_pool(name="t2", bufs=2) as t2pool,
            tc.tile_pool(name="acc", bufs=3) as apool,
        ):
            a_s = ppool.tile([P, 3], mybir.dt.float32)
            g_s = ppool.tile([P, 3], mybir.dt.float32)
            nc.sync.dma_start(out=a_s, in_=av)
            nc.sync.dma_start(out=g_s, in_=gv)

            for blk in range(n_blk):
                for fi in range(n_f):
                    xt = xpool.tile([P, f_tile], mybir.dt.float32)
                    nc.sync.dma_start(
                        out=xt,
                        in_=xT[blk * P : (blk + 1) * P, fi * f_tile : (fi + 1) * f_tile],
                    )
                    # t_k = relu(x + g_k)  on ScalarE (bias is per-partition)
                    acc = apool.tile([P, f_tile], mybir.dt.float32)
                    t1 = t1pool.tile([P, f_tile], mybir.dt.float32)
                    t2 = t2pool.tile([P, f_tile], mybir.dt.float32)
                    nc.scalar.activation(
                        acc, xt, mybir.ActivationFunctionType.Relu,
                        bias=g_s[:, 0:1], scale=1.0,
                    )
                    nc.scalar.activation(
                        t1, xt, mybir.ActivationFunctionType.Relu,
                        bias=g_s[:, 1:2], scale=1.0,
                    )
                    nc.scalar.activation(
                        t2, xt, mybir.ActivationFunctionType.Relu,
                        bias=g_s[:, 2:3], scale=1.0,
                    )
                    # acc = t0*a0 ; acc = t1*a1 + acc ; acc = t2*a2 + acc
                    nc.vector.tensor_scalar(
                        acc, acc, a_s[:, 0:1], None, mybir.AluOpType.mult
                    )
                    nc.vector.scalar_tensor_tensor(
                        acc, t1, a_s[:, 1:2], acc,
                        mybir.AluOpType.mult, mybir.AluOpType.add,
                    )
                    nc.vector.scalar_tensor_tensor(
                        acc, t2, a_s[:, 2:3], acc,
                        mybir.AluOpType.mult, mybir.AluOpType.add,
                    )
                    nc.sync.dma_start(
                        out=oT[blk * P : (blk + 1) * P, fi * f_tile : (fi + 1) * f_tile],
                        in_=acc,
                    )
    nc.compile()
    return nc


def _param_vecs(alpha: np.ndarray, gamma: np.ndarray):
    # av[p, k] = alpha[k, p % 64]
    av = np.ascontiguousarray(np.tile(alpha, (1, P // L)).T.astype(np.float32))
    gv = np.ascontiguousarray(np.tile(gamma, (1, P // L)).T.astype(np.float32))
    return av, gv


def kernel(x: np.ndarray, alpha: np.ndarray, gamma: np.ndarray) -> np.ndarray:
    from concourse.bass_utils import run_bass_kernel_spmd

    x = np.asarray(x, dtype=np.float32)
    av, gv = _param_vecs(np.asarray(alpha), np.asarray(gamma))

    xT = np.ascontiguousarray(x.T)  # [D, B]
    nc = build_program()
    in_maps = [
        {"xT": xT[c * DS : (c + 1) * DS], "av": av, "gv": gv} for c in range(N_CORES)
    ]
    res = run_bass_kernel_spmd(nc, in_maps, core_ids=list(range(N_CORES)))
    oT = np.concatenate([r["oT"] for r in res.results], axis=0)  # [D, B]
    return np.ascontiguousarray(oT.T)


# revision 14
# speedup vs baseline: 83798.4433x; 83798.4433x over previous
"""Trainium2 Bass kernel for nn_CustomActivation:

    out[b, d] = sum_k alpha[k, d % 64] * relu(x[b, d] + gamma[k, d % 64])

x: [8192, 4096] f32, alpha/gamma: [3, 64] f32.

Strategy
--------
Shard x along the FEATURE axis (columns) across 8 cores, 512 columns each,
and hand every core a TRANSPOSED, contiguous shard xT [512, 8192] (host-side
numpy transpose).  On-chip layout is then [partition = d, free = b]:

  * the per-column params alpha/gamma become per-PARTITION scalars
    ([128, 1] APs), so the whole op maps onto cheap per-partition hardware;
  * every DMA is fully contiguous (8KB runs per partition-row), full HBM BW.

Since the d-range of every 128-partition block is a multiple of 64, the
[128] param vectors (= param[k, p % 64]) are identical for all blocks/cores.

Math:
    t_k = relu(x + g_k)            [ScalarE activation, per-partition bias]
    acc = t_0 * a_0                [VectorE tensor_scalar, fp32 2x mode]
    acc = t_k * a_k + acc, k=1,2   [VectorE scalar_tensor_tensor, fused]

Engine budgets per core: DMA ~89us (32MB @ ~358GB/s HBM roofline),
ACT 3 relu passes ~88-91us, DVE ~88-91us -- measured ~88-91us/pass on HW,
i.e. at the roofline.  (A GPSIMD-assisted rebalance measured 134us/pass --
Pool elementwise adds are far slower on real HW than the cost model says.)

Params travel as one pv[128, 8] tensor: a0,a1,a2,g0,g1,-g2,a2*g2,g2.
"""

import numpy as np

import concourse.bacc as bacc
import concourse.mybir as mybir
from concourse.tile import TileContext

N_CORES = 8
B, D, L = 8192, 4096, 64
DS = D // N_CORES  # 512 feature columns per core
P = 128  # SBUF partitions


def build_program(ds: int = DS, b: int = B, f_tile: int = 2048, n_rep: int = 1):
    """Build the SPMD Bass program one core runs on its [ds, b] shard.

    n_rep > 1 repeats the whole pass (same inputs/outputs) for benchmarking.
    """
    nc = bacc.Bacc("TRN2", target_bir_lowering=False, debug=False)

    xT = nc.dram_tensor("xT", [ds, b], mybir.dt.float32, kind="ExternalInput").ap()
    pv = nc.dram_tensor("pv", [P, 8], mybir.dt.float32, kind="ExternalInput").ap()
    oT = nc.dram_tensor("oT", [ds, b], mybir.dt.float32, kind="ExternalOutput").ap()

    n_blk = ds // P
    n_f = b // f_tile
    A = mybir.AluOpType
    R = mybir.ActivationFunctionType.Relu

    with TileContext(nc) as tc:
        with (
            tc.tile_pool(name="params", bufs=1) as ppool,
            tc.tile_pool(name="xin", bufs=4) as xpool,
            tc.tile_pool(name="t1", bufs=2) as t1pool,
            tc.tile_pool(name="t2", bufs=2) as t2pool,
            tc.tile_pool(name="acc", bufs=3) as apool,
        ):
            p_s = ppool.tile([P, 8], mybir.dt.float32)
            nc.sync.dma_start(out=p_s, in_=pv)
            a0, a1, a2 = p_s[:, 0:1], p_s[:, 1:2], p_s[:, 2:3]
            g0, g1 = p_s[:, 3:4], p_s[:, 4:5]
            g2 = p_s[:, 7:8]

            for _rep in range(n_rep):
                for blk in range(n_blk):
                    for fi in range(n_f):
                        sl0 = slice(blk * P, (blk + 1) * P)
                        sl1 = slice(fi * f_tile, (fi + 1) * f_tile)
                        xt = xpool.tile([P, f_tile], mybir.dt.float32)
                        nc.sync.dma_start(out=xt, in_=xT[sl0, sl1])
                        t1 = t1pool.tile([P, f_tile], mybir.dt.float32)
                        t2 = t2pool.tile([P, f_tile], mybir.dt.float32)
                        acc = apool.tile([P, f_tile], mybir.dt.float32)
                        # ScalarE: t_k = relu(x + g_k)
                        nc.scalar.activation(acc, xt, R, bias=g0, scale=1.0)
                        nc.scalar.activation(t1, xt, R, bias=g1, scale=1.0)
                        nc.scalar.activation(t2, xt, R, bias=g2, scale=1.0)
                        # VectorE: acc = t0*a0; acc += t1*a1; acc += t2*a2
                        nc.vector.tensor_scalar(acc, acc, a0, None, A.mult)
                        nc.vector.scalar_tensor_tensor(
                            acc, t1, a1, acc, A.mult, A.add
                        )
                        nc.vector.scalar_tensor_tensor(
                            acc, t2, a2, acc, A.mult, A.add
                        )
                        nc.sync.dma_start(out=oT[sl0, sl1], in_=acc)
    nc.compile()
    return nc


def _param_vecs(alpha: np.ndarray, gamma: np.ndarray) -> np.ndarray:
    """pv[128, 8]: a0,a1,a2,g0,g1,-g2,a2*g2,g2 tiled (period L) along partitions."""
    a = np.tile(np.asarray(alpha, np.float32), (1, P // L))  # [3, 128]
    g = np.tile(np.asarray(gamma, np.float32), (1, P // L))
    pv = np.stack(
        [a[0], a[1], a[2], g[0], g[1], -g[2], a[2] * g[2], g[2]], axis=1
    )
    return np.ascontiguousarray(pv.astype(np.float32))


def kernel(x: np.ndarray, alpha: np.ndarray, gamma: np.ndarray) -> np.ndarray:
    from concourse.bass_utils import run_bass_kernel_spmd

    x = np.asarray(x, dtype=np.float32)
    pv = _param_vecs(alpha, gamma)

    xT = np.ascontiguousarray(x.T)  # [D, B]
    nc = build_program()
    in_maps = [
        {"xT": xT[c * DS : (c + 1) * DS], "pv": pv} for c in range(N_CORES)
    ]
    res = run_bass_kernel_spmd(nc, in_maps, core_ids=list(range(N_CORES)))
    oT = np.concatenate([r["oT"] for r in res.results], axis=0)  # [D, B]
    return np.ascontiguousarray(oT.T)


# revision 15
# speedup vs baseline: 86767.2352x; 1.0354x over previous
"""Trainium2 Bass kernel for nn_CustomActivation:

    out[b, d] = sum_k alpha[k, d % 64] * relu(x[b, d] + gamma[k, d % 64])

x: [8192, 4096] f32, alpha/gamma: [3, 64] f32.

Strategy
--------
Shard x along the FEATURE axis (columns) across 8 cores, 512 columns each,
and hand every core a TRANSPOSED, contiguous shard xT [512, 8192] (host-side
numpy transpose).  On-chip layout is then [partition = d, free = b]:

  * the per-column params alpha/gamma become per-PARTITION scalars
    ([128, 1] APs), so the whole op maps onto cheap per-partition hardware;
  * every DMA is fully contiguous (8KB runs per partition-row), full HBM BW.

Since the d-range of every 128-partition block is a multiple of 64, the
[128] param vectors (= param[k, p % 64]) are identical for all blocks/cores.

Math:
    t_k = relu(x + g_k)            [ScalarE activation, per-partition bias]
    acc = t_0 * a_0                [VectorE tensor_scalar, fp32 2x mode]
    acc = t_k * a_k + acc, k=1,2   [VectorE scalar_tensor_tensor, fused]

Engine budgets per core: DMA ~89us (32MB @ ~358GB/s HBM roofline),
ACT 3 relu passes ~88-91us, DVE ~88-91us -- measured ~88-91us/pass on HW,
i.e. at the roofline.  (A GPSIMD-assisted rebalance measured 134us/pass --
Pool elementwise adds are far slower on real HW than the cost model says.)

Params travel as one pv[128, 8] tensor: a0,a1,a2,g0,g1,-g2,a2*g2,g2.
"""

import numpy as np

import concourse.bacc as bacc
import concourse.mybir as mybir
from concourse.tile import TileContext

N_CORES = 8
B, D, L = 8192, 4096, 64
DS = D // N_CORES  # 512 feature columns per core
P = 128  # SBUF partitions


def build_program(ds: int = DS, b: int = B, f_tile: int = 2048, n_rep: int = 1):
    """Build the SPMD Bass program one core runs on its [ds, b] shard.

    n_rep > 1 repeats the whole pass (same inputs/outputs) for benchmarking.
    """
    nc = bacc.Bacc("TRN2", target_bir_lowering=False, debug=False)

    xT = nc.dram_tensor("xT", [ds, b], mybir.dt.float32, kind="ExternalInput").ap()
    pv = nc.dram_tensor("pv", [P, 8], mybir.dt.float32, kind="ExternalInput").ap()
    oT = nc.dram_tensor("oT", [ds, b], mybir.dt.float32, kind="ExternalOutput").ap()

    n_blk = ds // P
    n_f = b // f_tile
    A = mybir.AluOpType
    R = mybir.ActivationFunctionType.Relu

    with TileContext(nc) as tc:
        with (
            tc.tile_pool(name="params", bufs=1) as ppool,
            tc.tile_pool(name="xin", bufs=4) as xpool,
            tc.tile_pool(name="t1", bufs=2) as t1pool,
            tc.tile_pool(name="t2", bufs=2) as t2pool,
            tc.tile_pool(name="acc", bufs=3) as apool,
        ):
            p_s = ppool.tile([P, 8], mybir.dt.float32)
            nc.sync.dma_start(out=p_s, in_=pv)
            a0, a1, a2 = p_s[:, 0:1], p_s[:, 1:2], p_s[:, 2:3]
            g0, g1 = p_s[:, 3:4], p_s[:, 4:5]
            g2 = p_s[:, 7:8]

            for _rep in range(n_rep):
                for blk in range(n_blk):
                    for fi in range(n_f):
                        sl0 = slice(blk * P, (blk + 1) * P)
                        sl1 = slice(fi * f_tile, (fi + 1) * f_tile)
                        xt = xpool.tile([P, f_tile], mybir.dt.float32)
                        nc.sync.dma_start(out=xt, in_=xT[sl0, sl1])
                        t1 = t1pool.tile([P, f_tile], mybir.dt.float32)
                        t2 = t2pool.tile([P, f_tile], mybir.dt.float32)
                        acc = apool.tile([P, f_tile], mybir.dt.float32)
                        # ScalarE: t_k = relu(x + g_k)
                        nc.scalar.activation(acc, xt, R, bias=g0, scale=1.0)
                        nc.scalar.activation(t1, xt, R, bias=g1, scale=1.0)
                        nc.scalar.activation(t2, xt, R, bias=g2, scale=1.0)
                        # VectorE: acc = t0*a0; acc += t1*a1; acc += t2*a2
                        nc.vector.tensor_scalar(acc, acc, a0, None, A.mult)
                        nc.vector.scalar_tensor_tensor(
                            acc, t1, a1, acc, A.mult, A.add
                        )
                        nc.vector.scalar_tensor_tensor(
                            acc, t2, a2, acc, A.mult, A.add
                        )
                        nc.sync.dma_start(out=oT[sl0, sl1], in_=acc)
    nc.compile()
    return nc


def _param_vecs(alpha: np.ndarray, gamma: np.ndarray) -> np.ndarray:
    """pv[128, 8]: a0,a1,a2,g0,g1,-g2,a2*g2,g2 tiled (period L) along partitions."""
    a = np.tile(np.asarray(alpha, np.float32), (1, P // L))  # [3, 128]
    g = np.tile(np.asarray(gamma, np.float32), (1, P // L))
    pv = np.stack(
        [a[0], a[1], a[2], g[0], g[1], -g[2], a[2] * g[2], g[2]], axis=1
    )
    return np.ascontiguousarray(pv.astype(np.float32))


_program_cache: dict = {}


def kernel(x: np.ndarray, alpha: np.ndarray, gamma: np.ndarray) -> np.ndarray:
    from concourse.bass_utils import run_bass_kernel_spmd

    x = np.asarray(x, dtype=np.float32)
    pv = _param_vecs(alpha, gamma)

    xT = np.ascontiguousarray(x.T)  # [D, B]
    if "nc" not in _program_cache:
        _program_cache["nc"] = build_program()
    nc = _program_cache["nc"]
    in_maps = [
        {"xT": xT[c * DS : (c + 1) * DS], "pv": pv} for c in range(N_CORES)
    ]
    res = run_bass_kernel_spmd(nc, in_maps, core_ids=list(range(N_CORES)))
    oT = np.concatenate([r["oT"] for r in res.results], axis=0)  # [D, B]
    return np.ascontiguousarray(oT.T)
